# revision 40
# baseline (speedup 1.0000x reference)
"""Trainium2 Bass kernel: batched multi-head attention.

out[b,h] = softmax(Q[b,h] @ K[b,h].T / sqrt(D)) @ V[b,h]
with B=4, H=16, S=2048, D=64, fp32.

Sharding: the 64 (b,h) pairs are split across 8 NeuronCores, 8 pairs per
core; attention is independent per pair, so no cross-core communication.

Device dataflow per pair:
  1. Host pre-lays inputs:
       qt8 [128, 8, 512] fp8e4: split-precision Q^T — Q = Qh + Ql with
                        Qh = fp8(Q), Ql = fp8(Q - Qh); slot 0 = Qh^T
                        (duplicated in both partition halves), slot 1 =
                        Ql^T; dim1 = 2*qc + slot.
       kt8 [128, 32, 128] fp8e4: split K^T — partitions 0..63 = Kh^T,
                        64..127 = Kl^T, both slots identical;
                        dim1 = 2*ktile + slot.
       vo  [128, 1040] bf16: 16 chunks of [V_ktile | ones] width 65 —
                        the ones column yields the softmax denominator.
  2. scores^T[k,q]: ONE fp8 DoubleRow matmul per (k-tile, q-window)
     slice — the 2-slot structure contracts
       slot0: [Kh;Kl].[Qh;Qh] + slot1: [Kh;Kl].[Ql;Ql]
       = (Kh+Kl)x(Qh+Ql) = K x Q
     exactly (split residuals bound the representation error to
     ~|x|/256, measured 1.1e-3 rel on scores). DoubleRow costs 0.5
     cycles/output-row — HALF the fp32r cost: 131k PE cycles for all
     scores instead of 262k.
  3. P^T = exp(scores^T / 8), split across TWO engines:
       - ACT chunks: exact exp (scale folded), bf16 output.
       - DVE chunks: Schraudolph-style exp — one tensor_scalar
         computing int16(x * 128*log2e/8 + B) whose bit pattern IS
         bf16(exp(x/8)) up to the linear-mantissa interpolation error
         (~1.8% rms). B folds out the approximation's +3.97% mean log
         bias so DVE chunks carry no gain bias relative to ACT chunks
         (a uniform global gain would cancel in the softmax, a
         per-chunk one would not).
  4. PV is FLIPPED vs the naive layout: P^T [128k,128q] blocks are the
     stationary operand, V [128k, 65] bf16 the moving operand, so the
     output [128q, 65] uses all 128 PE rows and each matmul costs only
     65 cycles — half the PE time of the [65, 512] orientation.
     Accumulated over the 16 k-tiles in PSUM; the 65th column is the
     denominator. NOTE: start=True clears has_written for the WHOLE
     psum bank, so only the first matmul into each accumulator bank
     sets it.
  5. out blocks -> SBUF ob[128, 4, 260] -> HBM per q-window; host
     divides columns 0..63 by column 64. Out rows are already q-major.

Schedule: with fp8 scores the PE drops to ~110us busy and the TWO exp
engines become the joint bottleneck (~148.6us busy each; CoreSim e2e
157.5us). The exp stream is chunked [128, 1024] (2 PSUM banks per
chunk, 3 score buffers so each engine always has a buffered chunk —
wider 1536 chunks with only 2 buffers starve the engines on the slot
ping-pong and regress badly). Chunks alternate ACT/DVE 5:4 (strict
alternation keeps each engine's serial chain short; the ratio balances
1038ns ACT vs 1192ns DVE chunks + 12.7us of DVE drain copies, and
keeps the approximated fraction at 44% -> 9.6e-3 L2 error, budget
2e-2). A deep 12-chunk software pipeline (PV for chunk c emitted after
scores for chunk c+12) decouples PE from exp jitter. PSUM: 3x2 banks
scores + 2x1 bank PV accumulators = 8. Input DMAs split across the SP
HWDGE ring and SWDGE (gpsimd), ordered by first need. Measured
regressions (do not retry): drain alternation to ACT, batched
two-window drains with single-buffered o65, asymmetric 1536/1024 slot
rotations, runt edge chunks, tail exp splitting, SWDGE startup/tail
DMA rerouting.
"""

import sys

sys.path.insert(0, "/opt/trn_rl_repo")

import numpy as np

import concourse.bacc as bacc
import concourse.bass as bass
import concourse.mybir as mybir
from concourse.bass_utils import run_bass_kernel_spmd
from concourse.tile import TileContext

B, H, S, D = 4, 16, 2048, 64
N_CORES = 8
PAIRS = B * H              # 64 independent (b, h) attention problems
PPC = PAIRS // N_CORES     # 8 pairs per core
KT = S // 128              # 16 k-tiles of 128 rows
QC = 512                   # q-chunk width (4 windows of 512)
CW = 1024                  # exp chunk width (2 score slices of 512)
F32 = mybir.dt.float32
F32R = mybir.dt.float32r
BF16 = mybir.dt.bfloat16
I16 = mybir.dt.int16
F8 = mybir.dt.float8e4
EXP = mybir.ActivationFunctionType.Exp
SCALE = 1.0 / np.sqrt(D)   # folded into the activation / Schraudolph A

# Schraudolph constants for bf16 bit layout (1-8-7):
#   bits16(exp(s/8)) ~= round(128 * (127 + (s/8) * log2(e)))
A16 = 128.0 * float(np.log2(np.e)) * SCALE   # = 16*log2(e) = 23.0831...
# +0.5 rounds under truncation; -7.334 folds out the mean log error of the
# linear-mantissa interpolation (+3.97%) so DVE chunks carry no gain bias
# relative to the exact ACT chunks (a uniform global gain would cancel in
# the softmax, a per-chunk one would not).
B16 = 128.0 * 127.0 + 0.5 - 7.334


def build_bass(pattern="ADADADADA", pv_lag=12, swdge_first=False,
               split_last_drain=False, tail_split=0, cw=1024, sc_bufs=3,
               last_dma_swdge=False, drain_alt=False, slots=None):
    """pattern: per-period chunk assignment, 'A' = ACT exact exp, 'D' =
    DVE Schraudolph exp. Strict alternation keeps each engine's chain
    short so neither serializes against the PE-paced pipeline.
    pv_lag is the software-pipeline depth: PV matmuls for chunk c are
    emitted after the scores matmuls for chunk c+pv_lag, giving the exp
    engines ~pv_lag PE-chunk-times of slack before PE needs their
    output."""
    period = len(pattern)
    nc = bacc.Bacc()
    qt_d = nc.declare_dram_parameter("qt8", [PPC, 128, 8, QC], F8, isOutput=False)
    kt_d = nc.declare_dram_parameter("kt8", [PPC, 128, 2 * KT, 128], F8, isOutput=False)
    vo_d = nc.declare_dram_parameter("vo", [PPC, 128, KT * 65], BF16, isOutput=False)
    out_d = nc.declare_dram_parameter("ot", [PPC, 128, KT * 65], F32, isOutput=True)

    with TileContext(nc) as tc:
        with (
            tc.tile_pool(name="qt", bufs=2) as qt_pool,
            tc.tile_pool(name="kt", bufs=2) as kt_pool,
            tc.tile_pool(name="vo", bufs=2) as vo_pool,
            tc.tile_pool(name="pt", bufs=pv_lag + 2) as pt_pool,
            tc.tile_pool(name="ob", bufs=2) as ob_pool,
            tc.tile_pool(name="ps_s", bufs=(1 if slots else sc_bufs), space="PSUM") as ps_s_pool,
            tc.tile_pool(name="ps_m", bufs=2, space="PSUM") as ps_m_pool,
            tc.tile_pool(name="ps_o", bufs=(1 if slots else 2), space="PSUM") as ps_o_pool,
        ):
            # Stream of 512-wide scores^T slices, pair-major, then q
            # window, then k-tile. 2 consecutive slices = one exp chunk;
            # 16 slices (8 chunks) = one (pair, q-window) PV accumulation
            # group, so chunk and window boundaries always align.
            stream = [
                (p, qc, t)
                for p in range(PPC)
                for qc in range(S // QC)
                for t in range(KT)
            ]
            if slots:
                chunks = []
                pos = 0
                while pos < len(stream):
                    w = slots[len(chunks) % len(slots)]
                    chunks.append(stream[pos : pos + w])
                    pos += w
            else:
                nsl = cw // 512
                chunks = [stream[i : i + nsl] for i in range(0, len(stream), nsl)]
            # Engine assignment: strict-ish alternation with load-aware
            # corrections (chains <= 2; cumulative engine loads kept within
            # ~one chunk). ACT is exact exp, DVE approximated, so ties
            # prefer ACT.
            if slots:
                assign = []
                tA = tD = 0.0
                for c in chunks:
                    w5 = len(c) * 512
                    cA = (w5 + 222) / 1.2
                    cD = (w5 + 120) / 0.96
                    nxt = "A" if (not assign or assign[-1] == "D") else "D"
                    if nxt == "A" and tA + cA > tD + 800:
                        nxt = "D"
                    elif nxt == "D" and tD + cD > tA + 800:
                        nxt = "A"
                    if len(assign) >= 2 and assign[-1] == assign[-2] == nxt:
                        nxt = "A" if nxt == "D" else "D"
                    if nxt == "A":
                        tA += cA
                    else:
                        tD += cD
                    assign.append(nxt)
            else:
                assign = [pattern[ci % period] for ci in range(len(chunks))]
            tiles = {}   # pair -> (qt, kt, vo, ob)
            o65s = {}    # (pair, qc) -> psum accumulator [128, 4*65]
            pts = {}     # chunk idx -> pt tile (bf16 view of P^T)

            def emit_pv(ci):
                pt = pts.pop(ci)
                for i, (p, qc, t) in enumerate(chunks[ci]):
                    o65 = o65s[(p, qc)]
                    vo, ob = tiles[p][2], tiles[p][3]
                    for qb in range(4):
                        # start=True clears has_written for the WHOLE psum
                        # bank, so only the very first matmul into the bank
                        # may set it; the other qb groups' first write lands
                        # on has_written=0 elements and overwrites (the
                        # per-element accumulate-or-overwrite semantics).
                        nc.tensor.matmul(
                            o65[:, qb * 65 : (qb + 1) * 65],
                            pt[:, i * 512 + qb * 128 : i * 512 + (qb + 1) * 128],
                            vo[:, t * 65 : (t + 1) * 65],
                            start=(t == 0 and qb == 0),
                            stop=(t == KT - 1),
                            skip_group_check=True,
                        )
                    if t == KT - 1:
                        nc.vector.tensor_copy(
                            out=ob[:, qc, :], in_=o65[:]
                        )
                        del o65s[(p, qc)]
                        # Stream each q-window out as soon as drained so
                        # the kernel tail only carries the final window.
                        nc.sync.dma_start(
                            out=out_d[p][:, qc * 260 : (qc + 1) * 260],
                            in_=ob[:, qc, :],
                        )

            for ci, chunk in enumerate(chunks):
                if slots:
                    if len(chunk) == 3:
                        sc = ps_s_pool.tile([128, 1536], F32, tag="s")
                    else:
                        sc = ps_m_pool.tile([128, 1024], F32, tag="m")
                else:
                    sc = ps_s_pool.tile([128, cw], F32, tag="s")
                for i, (p, qc, t) in enumerate(chunk):
                    if p not in tiles:
                        # Stage DMAs so the first scores matmul's operands
                        # (kt cols 0:128, qt cols 0:512) land first.
                        # Two DMA issue paths in parallel, each ordered by
                        # first need: SP HWDGE carries the scores-critical
                        # pieces, SWDGE (gpsimd) the bulk remainders.
                        # SWDGE (gpsimd) has the shortest first-byte latency
                        # (25ns issue + 994 fixed vs 565+1275 on the SP
                        # HWDGE ring), so the two pieces the first scores
                        # matmul waits on go there, on separate queues.
                        kt = kt_pool.tile([128, 2 * KT, 128], F8)
                        qt = qt_pool.tile([128, 8, QC], F8)
                        nc.sync.dma_start(out=kt[:, 0:4, :], in_=kt_d[p][:, 0:4, :])
                        nc.gpsimd.dma_start(out=qt[:, 0:2, :], in_=qt_d[p][:, 0:2, :])
                        vo = vo_pool.tile([128, KT * 65], BF16)
                        nc.gpsimd.dma_start(out=vo[:], in_=vo_d[p])
                        nc.gpsimd.dma_start(
                            out=kt[:, 4 : 2 * KT, :], in_=kt_d[p][:, 4 : 2 * KT, :]
                        )
                        nc.sync.dma_start(out=qt[:, 2:4, :], in_=qt_d[p][:, 2:4, :])
                        nc.gpsimd.dma_start(out=qt[:, 4:8, :], in_=qt_d[p][:, 4:8, :])
                        ob = ob_pool.tile([128, 4, 260], F32)  # [qc, 260]
                        tiles[p] = (qt, kt, vo, ob)
                    qt, kt = tiles[p][0], tiles[p][1]
                    if (p, qc) not in o65s:
                        o65s[(p, qc)] = ps_o_pool.tile(
                            [128, 4 * 65], F32, name="o65", tag="o65"
                        )
                    nc.tensor.matmul(
                        sc[:, i * 512 : (i + 1) * 512],
                        kt[:, 2 * t : 2 * t + 2, :],
                        qt[:, 2 * qc : 2 * qc + 2, :],
                        start=True,
                        stop=True,
                        perf_mode=mybir.MatmulPerfMode.DoubleRow,
                    )
                w = len(chunk) * 512
                pt = pt_pool.tile([128, 1536 if slots else cw], BF16, tag="p")
                if tail_split and ci >= len(chunks) - tail_split and w > 512:
                    # Pipeline drain: split the final chunks' exp across
                    # both engines so the tail backlog clears ~2x faster.
                    nc.scalar.activation(pt[:, 0:512], sc[:, 0:512], EXP, scale=SCALE)
                    nc.vector.tensor_scalar(
                        out=pt[:, 512:w].bitcast(I16),
                        in0=sc[:, 512:w],
                        scalar1=A16,
                        scalar2=B16,
                        op0=mybir.AluOpType.mult,
                        op1=mybir.AluOpType.add,
                    )
                elif assign[ci] == "A":
                    nc.scalar.activation(pt[:, :w], sc[:, :w], EXP, scale=SCALE)
                else:
                    nc.vector.tensor_scalar(
                        out=pt[:, :w].bitcast(I16),
                        in0=sc[:, :w],
                        scalar1=A16,
                        scalar2=B16,
                        op0=mybir.AluOpType.mult,
                        op1=mybir.AluOpType.add,
                    )
                pts[ci] = pt
                if ci >= pv_lag:
                    emit_pv(ci - pv_lag)
            for ci in range(len(chunks) - pv_lag, len(chunks)):
                emit_pv(ci)
    nc.compile()
    return nc


def _prep_inputs(query, key, value):
    """Host-side layout prep. Returns per-core input maps."""
    q = np.ascontiguousarray(query.reshape(PAIRS, S, D))
    k = np.ascontiguousarray(key.reshape(PAIRS, S, D))
    v = np.ascontiguousarray(value.reshape(PAIRS, S, D))

    f8 = mybir.dt.np(F8)

    def split8(x):
        xh = x.astype(f8)
        xl = (x - xh.astype(np.float32)).astype(f8)
        return xh, xl

    # Split-precision fp8 layouts for the DoubleRow scores matmul.
    # qt8[p, part, 2*qc+slot, n]: partitions duplicate the d rows in both
    # halves; slot 0 = Qh^T, slot 1 = Ql^T.
    qt_t = np.ascontiguousarray(q.transpose(0, 2, 1))      # [PAIRS, 64, S]
    qh, ql = split8(qt_t)
    qhr = qh.reshape(PAIRS, D, 4, QC)
    qlr = ql.reshape(PAIRS, D, 4, QC)
    qt8 = np.empty((PAIRS, 128, 4, 2, QC), dtype=f8)
    qt8[:, :D, :, 0] = qhr
    qt8[:, D:, :, 0] = qhr
    qt8[:, :D, :, 1] = qlr
    qt8[:, D:, :, 1] = qlr
    qt8 = qt8.reshape(PAIRS, 128, 8, QC)

    # kt8[p, part, 2*t+slot, m]: partitions 0..63 = Kh^T, 64..127 = Kl^T;
    # both slots identical (the slot sum contracts against Qh then Ql).
    kt_t = np.ascontiguousarray(k.transpose(0, 2, 1))      # [PAIRS, 64, S]
    kh, kl = split8(kt_t)
    khr = kh.reshape(PAIRS, D, KT, 128)
    klr = kl.reshape(PAIRS, D, KT, 128)
    kt8 = np.empty((PAIRS, 128, KT, 2, 128), dtype=f8)
    kt8[:, :D, :, 0] = khr
    kt8[:, D:, :, 0] = klr
    kt8[:, :D, :, 1] = khr
    kt8[:, D:, :, 1] = klr
    kt8 = kt8.reshape(PAIRS, 128, 2 * KT, 128)

    vt = v.reshape(PAIRS, KT, 128, D).transpose(0, 2, 1, 3)  # [PAIRS,128,KT,64]
    vo = np.empty((PAIRS, 128, KT, 65), dtype=np.float32)
    vo[:, :, :, :D] = vt
    vo[:, :, :, D] = 1.0
    vo = vo.reshape(PAIRS, 128, KT * 65).astype(mybir.dt.np(BF16))

    in_maps = []
    for c in range(N_CORES):
        sl = slice(c * PPC, (c + 1) * PPC)
        in_maps.append(
            {
                "qt8": np.ascontiguousarray(qt8[sl]),
                "kt8": np.ascontiguousarray(kt8[sl]),
                "vo": np.ascontiguousarray(vo[sl]),
            }
        )
    return in_maps


_CACHED_NC = None


def kernel(query, key, value, _want_results_obj=False, _trace=False):
    global _CACHED_NC
    if _CACHED_NC is None:
        _CACHED_NC = build_bass()
    nc = _CACHED_NC

    in_maps = _prep_inputs(query, key, value)
    res = run_bass_kernel_spmd(
        nc, in_maps, core_ids=list(range(N_CORES)), trace=_trace
    )

    ot = np.concatenate(
        [np.asarray(res.results[c]["ot"]) for c in range(N_CORES)], axis=0
    ).astype(np.float32)
    # ot[p] is [128 part, 16 qb, 65]; q = qb*128 + part; col 64 = denom.
    ot = ot.reshape(PAIRS, 128, KT, 65).transpose(0, 2, 1, 3)
    ot = ot.reshape(PAIRS, S, 65)
    out = ot[:, :, :D] / ot[:, :, D : D + 1]
    out = out.reshape(B, H, S, D).astype(np.float32)
    if _want_results_obj:
        return out, res
    return out


if __name__ == "__main__":
    rng = np.random.default_rng(0)
    q = rng.standard_normal((B, H, S, D), dtype=np.float32)
    k = rng.standard_normal((B, H, S, D), dtype=np.float32)
    v = rng.standard_normal((B, H, S, D), dtype=np.float32)
    o = kernel(query=q, key=k, value=v)
    print("out shape:", o.shape, o.dtype)
